# revision 35
# baseline (speedup 1.0000x reference)
"""AFT (attention-free transformer) block kernel for 8 Trainium2 NeuronCores.

Sharding: batch b in 0..3 -> core pair (2b, 2b+1); each core handles 4096
contiguous tokens of that batch's 8192-token sequence.  The only cross-core
dependency is the cumsum carry (per-channel totals of w=exp(k_norm) and
kv=w*v over the first half), exchanged with a per-pair AllGather; odd cores
apply the received carry, even cores multiply it by a 0 mask.

Layout: everything is [token=partition, channel=free].  Matmuls run in bf16
(inputs pre-transposed).  The per-128-token cumsum is a lower-triangular
matmul on the PE; the running carry stays fp32 and already broadcast across
partitions: an all-ones matmul yields the tile's column-sum replicated on
every partition, so the carry update is a single vector add per tile.
"""

import sys
import numpy as np
import ml_dtypes

for _p in ("/opt/trn_rl_repo",):
    if _p not in sys.path:
        sys.path.insert(0, _p)

P = 128
D = 1024
N_CORES = 8
B_FULL, T_FULL = 4, 8192
CHUNK = T_FULL // 2          # tokens per core
NT_FULL = CHUNK // P         # 32 tiles per core
RMS_EPS = 1.1920929e-07
AFT_EPS = 1e-6

_nc_cache = {}
USE_FP8 = True

_ACT_TABLES_PATCHED = False


def _restrict_act_tables():
    # Confine activation-table choice to two sets (phase A: ln/exp/square,
    # phase B: sigmoid) so the ACT engine loads each table once instead of
    # thrashing between per-function tables. Ids (dict order) are preserved;
    # emptied sets are merely unchoosable.
    global _ACT_TABLES_PATCHED
    if _ACT_TABLES_PATCHED:
        return
    import concourse.bacc as bacc_mod

    keep = {"natural_log_exp_and_others", "silu_and_others"}
    orig = bacc_mod.get_activation_tables

    def restricted(arch, _orig=orig, _keep=keep):
        return {
            name: (funcs if name in _keep else set())
            for name, funcs in _orig(arch).items()
        }

    bacc_mod.get_activation_tables = restricted
    _ACT_TABLES_PATCHED = True


def build_nc(n_tiles=NT_FULL, num_devices=N_CORES, use_collective=True, use_fp8=True):
    import concourse.mybir as mybir
    import concourse.tile as tile
    from concourse import bacc

    AF = mybir.ActivationFunctionType
    fp32 = mybir.dt.float32
    bf16 = mybir.dt.bfloat16
    f8 = mybir.dt.float8e4 if use_fp8 else mybir.dt.bfloat16
    DR = mybir.MatmulPerfMode.DoubleRow if use_fp8 else None
    chunk = n_tiles * P

    _restrict_act_tables()
    nc = bacc.Bacc(
        "TRN2",
        target_bir_lowering=False,
        debug=False,
        enable_asserts=False,
        num_devices=num_devices,
    )

    x_d = nc.dram_tensor("x", [chunk, D], fp32, kind="ExternalInput")
    wqkv_d = nc.dram_tensor("wqkvT", [D, 3 * D], f8, kind="ExternalInput")
    wsw_d = nc.dram_tensor("wswT", [D, 2 * D], f8, kind="ExternalInput")
    wout_d = nc.dram_tensor("woutT", [D, D], f8, kind="ExternalInput")
    tri_d = nc.dram_tensor("triT", [P, P], bf16, kind="ExternalInput")
    mask_d = nc.dram_tensor("cmask", [2, 1], fp32, kind="ExternalInput")
    out_d = nc.dram_tensor("out", [chunk, D], fp32, kind="ExternalOutput")

    x_t = x_d.ap().rearrange("(n p) d -> n p d", p=P)
    out_t = out_d.ap().rearrange("(n p) d -> n p d", p=P)

    H = D // 2  # 512, matmul free-dim chunk

    with tile.TileContext(nc) as tc:
        with (
            tc.tile_pool(name="consts", bufs=1) as consts,
            tc.tile_pool(name="wbl", bufs=3) as wbl,
            tc.tile_pool(name="dram", bufs=1, space="DRAM") as dram,
        ):
            # ---- persistent constants in SBUF ----
            tri_sb = consts.tile([P, P], bf16)
            nc.sync.dma_start(tri_sb[:], tri_d.ap())
            ones_col = consts.tile([1, P], bf16)
            nc.any.memset(ones_col[:], 1.0)
            mask_sb = consts.tile([2, 1], fp32)
            nc.sync.dma_start(mask_sb[:], mask_d.ap())
            eps_sb = consts.tile([P, 1], fp32)
            nc.any.memset(eps_sb[:], RMS_EPS)
            ln2_sb = consts.tile([P, 1], fp32)
            nc.any.memset(ln2_sb[:], float(np.log(2.0)))

            # ---- DRAM scratch for phase A -> B ----
            wcum_dram = dram.tile([n_tiles, P, D], bf16)
            kvcum_dram = dram.tile([n_tiles, P, D], bf16)
            sigq_dram = dram.tile([n_tiles, P, D], bf16)
            cc_in = dram.tile([2, D], fp32)
            cc_out = dram.tile([4, D], fp32)

            # phase-B weights get a dedicated pool that coexists with phase A
            # so their SWDGE loads overlap phase A instead of waiting on a
            # WAR-reused SBUF range at the phase boundary.
            wsw_sb = consts.tile([P, 8, 2 * D], f8)
            wout_sb = consts.tile([P, 8, D], f8)

            # =========================== PHASE A ===========================
            with (
                tc.tile_pool(name="ps_qkv", bufs=4, space="PSUM") as ps_qkv,
                tc.tile_pool(name="ps_scan", bufs=2, space="PSUM") as ps_scan,
                tc.tile_pool(name="wka", bufs=3) as wk,
                tc.tile_pool(name="cbp", bufs=2) as cbp,
                tc.tile_pool(name="wqa", bufs=1) as wqa,
            ):
                wq_ap = wqkv_d.ap().rearrange("(ko p) n -> p ko n", p=P)
                wq_sb = wqa.tile([P, 8, 3 * D], f8, name="wq_sb")
                for kk in range(8):
                    nc.gpsimd.dma_start(wq_sb[:, kk, :], wq_ap[:, kk, :])
                wsw_ap = wsw_d.ap().rearrange("(ko p) n -> p ko n", p=P)
                wout_ap = wout_d.ap().rearrange("(ko p) n -> p ko n", p=P)
                bweight_dmas = []
                for kk in range(8):
                    bweight_dmas.append(
                        nc.gpsimd.dma_start(wsw_sb[:, kk, :], wsw_ap[:, kk, :])
                    )
                    bweight_dmas.append(
                        nc.gpsimd.dma_start(wout_sb[:, kk, :], wout_ap[:, kk, :])
                    )

                # inter-tile carry rows: (tile, row) with row 127 of the
                # previous spilled cum tile (an AP slice, no copy); zero
                # rows for tile 0
                cb = {}
                for t in ("w", "kv"):
                    zrow = cbp.tile([1, D], bf16, tag=f"cb_{t}", name=f"cb_{t}")
                    nc.any.memzero(zrow[:])
                    cb[t] = (zrow, 0)

                for i in range(n_tiles):
                    # x as bf16 via casting SWDGE DMA.  rms_norm is scale-
                    # invariant per token, so q/k need no normalized x at
                    # all; only v needs the rs scale (folded into the kv
                    # mul below).
                    xb = wk.tile([P, D], bf16, tag="xt", bufs=3)
                    nc.gpsimd.dma_start(xb[:], x_t[i])

                    sq = wk.tile([P, D], fp32, tag="sqscratch", bufs=2)
                    ssq = wk.tile([P, 1], fp32, tag="ssq")
                    nc.scalar.activation(sq[:], xb[:], AF.Square, accum_out=ssq[:])
                    # rsqrt via exp(-0.5*ln(mean+eps)): stays in the ln/exp table
                    lms = wk.tile([P, 1], fp32, tag="lms")
                    nc.scalar.activation(
                        lms[:], ssq[:], AF.Ln, scale=1.0 / D, bias=eps_sb[:]
                    )
                    rs = wk.tile([P, 1], fp32, tag="rs")
                    nc.scalar.activation(rs[:], lms[:], AF.Exp, scale=-0.5)

                    # transpose x for matmul lhsT (single xbar DMA)
                    xnT = wk.tile([P, 8, P], bf16, tag="xnT", bufs=2)
                    nc.sync.dma_start_transpose(xnT[:], xb[:])
                    if use_fp8:
                        xnT8 = wk.tile([P, 8, P], f8, tag="xnT8", bufs=2)
                        nc.gpsimd.tensor_copy(xnT8[:], xnT[:])
                    else:
                        xnT8 = xnT

                    # qkv chunk-pair matmul: chunks (c0, c0+1) of 6x512
                    def mm_pair(c0):
                        pair = [
                            ps_qkv.tile([P, H], fp32, tag="qkv", name=f"qkv{c0}_{c}")
                            for c in range(2)
                        ]
                        nk = 4 if use_fp8 else 8
                        for m in range(nk):
                            ms = slice(2 * m, 2 * m + 2) if use_fp8 else m
                            for ci in range(2):
                                nc.tensor.matmul(
                                    pair[ci][:],
                                    lhsT=xnT8[:, ms, :],
                                    rhs=wq_sb[
                                        :, ms, (c0 + ci) * H : (c0 + ci + 1) * H
                                    ],
                                    start=(m == 0),
                                    stop=(m == nk - 1),
                                    perf_mode=DR,
                                )
                        return pair

                    def rms_scale(pair, nm):
                        sq2 = wk.tile([P, H], fp32, tag="sqscratch", name=f"sq2_{nm}", bufs=2)
                        pa = wk.tile([P, 1], fp32, tag=f"pa_{nm}", name=f"pa_{nm}")
                        pb = wk.tile([P, 1], fp32, tag=f"pb_{nm}", name=f"pb_{nm}")
                        nc.scalar.activation(
                            sq2[:], pair[0][:], AF.Square, accum_out=pa[:]
                        )
                        nc.scalar.activation(
                            sq2[:], pair[1][:], AF.Square, accum_out=pb[:]
                        )
                        st = wk.tile([P, 1], fp32, tag=f"st_{nm}", name=f"st_{nm}")
                        nc.vector.tensor_add(st[:], pa[:], pb[:])
                        nc.scalar.activation(
                            st[:], st[:], AF.Ln, scale=1.0 / D, bias=eps_sb[:]
                        )
                        rr = wk.tile([P, 1], fp32, tag=f"rr_{nm}", name=f"rr_{nm}")
                        nc.scalar.activation(rr[:], st[:], AF.Exp, scale=-0.5)
                        return rr

                    # k chunks -> w = exp(rms(k))
                    kp = mm_pair(2)
                    rsk = rms_scale(kp, "k")
                    w_sb = wk.tile([P, D], bf16, tag="w_sb", bufs=2)
                    # w' = 2*exp(rms(k)) = exp(rms(k) + ln2): the doubling
                    # makes 1/(w'cum+2eps) = 0.5/(wcum+eps), absorbing the
                    # 0.5 of the phase-B tanh sigmoid for free
                    for j in range(2):
                        js = slice(j * H, (j + 1) * H)
                        nc.scalar.activation(
                            w_sb[:, js], kp[j][:], AF.Exp, scale=rsk[:],
                            bias=ln2_sb[:],
                        )

                    # q chunks -> rms(q) spilled (tanh sigmoid in phase B)
                    qp = mm_pair(0)
                    rsq = rms_scale(qp, "q")
                    sigq = wk.tile([P, D], bf16, tag="sigq", bufs=2)
                    for j in range(2):
                        js = slice(j * H, (j + 1) * H)
                        nc.scalar.activation(
                            sigq[:, js], qp[j][:], AF.Copy, scale=rsq[:]
                        )
                    nc.sync.dma_start(sigq_dram[i], sigq[:])

                    # v chunks -> kv = w * (rs * v): the x rms scale rides here
                    vp = mm_pair(4)
                    kv_sb = wk.tile([P, D], bf16, tag="kv_sb", bufs=2)
                    for j in range(2):
                        js = slice(j * H, (j + 1) * H)
                        nc.vector.scalar_tensor_tensor(
                            kv_sb[:, js], vp[j][:], rs[:], w_sb[:, js],
                            mybir.AluOpType.mult, mybir.AluOpType.mult,
                        )

                    # chunked causal cumsum; the inter-tile carry is row 127
                    # of the previous cum tile, folded into the scan PSUM via
                    # a rank-1 matmul.
                    for t, src, dst, ceng in (
                        ("w", w_sb, wcum_dram, "act"),
                        ("kv", kv_sb, kvcum_dram, "dve"),
                    ):
                        ps = ps_scan.tile([P, D], fp32, tag="scan", name=f"scan_{t}")
                        for j in range(2):
                            js = slice(j * H, (j + 1) * H)
                            nc.tensor.matmul(
                                ps[:, js], lhsT=tri_sb[:], rhs=src[:, js],
                                start=True, stop=False,
                            )
                        cbt, cbr = cb[t]
                        for j in range(2):
                            js = slice(j * H, (j + 1) * H)
                            nc.tensor.matmul(
                                ps[:, js], lhsT=ones_col[:],
                                rhs=cbt[cbr : cbr + 1, js],
                                start=False, stop=True,
                            )
                        cum = wk.tile([P, D], bf16, tag=f"cum_{t}", name=f"cum_{t}", bufs=3)
                        if ceng == "act":
                            nc.scalar.copy(cum[:], ps[:])
                        else:
                            nc.vector.tensor_copy(cum[:], ps[:])
                        nc.sync.dma_start(dst[i], cum[:])
                        # matmul operands must sit at partition 0/32/64, so
                        # hop row 127 down via a tiny SWDGE sbuf->sbuf DMA
                        nxt = cbp.tile([1, D], bf16, tag=f"cb_{t}", name=f"cbn_{t}")
                        nc.gpsimd.dma_start(nxt[:], cum[127:128, :])
                        cb[t] = (nxt, 0)
                        if i + 1 == n_tiles:
                            row = 0 if t == "w" else 1
                            nc.gpsimd.dma_start(
                                cc_in[row : row + 1, :], cum[127:128, :]
                            )

            # ======================= carry exchange ========================
            import concourse.mybir as _mybir

            gath = consts.tile([2, D], fp32)
            if use_collective:
                nc.gpsimd.collective_compute(
                    "AllGather",
                    _mybir.AluOpType.bypass,
                    replica_groups=[[2 * p, 2 * p + 1] for p in range(num_devices // 2)],
                    ins=[cc_in[:].opt()],
                    outs=[cc_out[:].opt()],
                    cc_dim="Partition",
                )
                nc.sync.dma_start(gath[:], cc_out[0:2, :])
            else:
                nc.any.memzero(gath[:])

            gathm = consts.tile([2, D], fp32)
            nc.vector.tensor_scalar_mul(gathm[:], gath[:], mask_sb[:])
            row1 = consts.tile([1, D], fp32)
            nc.sync.dma_start(row1[:], gathm[1:2, :])
            cwb32 = consts.tile([P, D], fp32)
            ckb32 = consts.tile([P, D], fp32)
            nc.gpsimd.partition_broadcast(cwb32[:], gathm[0:1, :])
            nc.gpsimd.partition_broadcast(ckb32[:], row1[:])
            cwb = consts.tile([P, D], bf16)
            ckb = consts.tile([P, D], bf16)
            # fold the denominator epsilon into the w-carry tile (2x because
            # the spilled w-cumsums carry the 2*exp(k) doubling)
            nc.vector.tensor_scalar_add(cwb[:], cwb32[:], 2 * AFT_EPS)
            nc.vector.tensor_copy(ckb[:], ckb32[:])

            # =========================== PHASE B ===========================
            with (
                tc.tile_pool(name="ps_uv", bufs=5, space="PSUM") as ps_uv,
                tc.tile_pool(name="ps_o", bufs=3, space="PSUM") as ps_o,
                tc.tile_pool(name="wkb", bufs=4) as wb,
            ):
                prev = None  # deferred (pairs, h, i) consumed one step later
                for i in range(n_tiles + 1):
                    if i < n_tiles:
                        # --- y chain for tile i ---
                        # y2 = sigmoid(q)*kvc/wc with sigmoid(q) =
                        # 0.5*(tanh(q/2)+1): yt = (0.5*kc+ckb/2)/twc, then
                        # y2 = (tanh(q/2)+1)*yt.
                        wc = wbl.tile([P, D], bf16, tag="wc", bufs=3)
                        nc.sync.dma_start(wc[:], wcum_dram[i])
                        kc = wbl.tile([P, D], bf16, tag="kc", bufs=3)
                        nc.sync.dma_start(kc[:], kvcum_dram[i])
                        sgq = wbl.tile([P, D], bf16, tag="sgq", bufs=3)
                        nc.sync.dma_start(sgq[:], sigq_dram[i])
                        sig = wb.tile([P, D], bf16, tag="sig")
                        nc.scalar.activation(sig[:], sgq[:], AF.Tanh, scale=0.5)

                        twc = wb.tile([P, D], bf16, tag="twc")
                        nc.gpsimd.tensor_add(twc[:], wc[:], cwb[:])
                        rec = wb.tile([P, D], bf16, tag="rec")
                        with nc.allow_low_precision(reason="y denominators are bf16 anyway"):
                            nc.vector.reciprocal(rec[:], twc[:])
                        tkc = wb.tile([P, D], bf16, tag="tkc")
                        nc.gpsimd.tensor_add(tkc[:], kc[:], ckb[:])
                        yt = wb.tile([P, D], bf16, tag="yt")
                        nc.vector.tensor_mul(yt[:], tkc[:], rec[:])
                        y2 = wb.tile([P, D], bf16, tag="y2")
                        nc.vector.scalar_tensor_tensor(
                            y2[:], sig[:], 1.0, yt[:],
                            mybir.AluOpType.add, mybir.AluOpType.mult,
                        )
                        y2T = wb.tile([P, 8, P], bf16, tag="y2T")
                        nc.sync.dma_start_transpose(y2T[:], y2[:])
                        if use_fp8:
                            y2T8 = wb.tile([P, 8, P], f8, tag="y2T8")
                            nc.gpsimd.tensor_copy(y2T8[:], y2T[:])
                        else:
                            y2T8 = y2T

                    # --- previous tile's silu/h consumption (frees uv psums) ---
                    if prev is not None:
                        pairs_p, h_p, ip = prev
                        for j, (pu, pg) in enumerate(pairs_p):
                            js = slice(j * H, (j + 1) * H)
                            # weights are host-scaled 16x, y2 carries 32x
                            # (16x weights + the 2x w-doubling), so pg/pu
                            # are 512x; h is kept at 32x so its fp8 cast
                            # stays clear of e4m3 subnormals yet under 448
                            sl = wb.tile([P, H], bf16, tag="sl", name=f"sl{j}")
                            nc.scalar.activation(
                                sl[:], pg[:], AF.Silu, scale=1.0 / 512.0
                            )
                            nc.vector.scalar_tensor_tensor(
                                h_p[:, js], pu[:], 1.0 / 16.0, sl[:],
                                mybir.AluOpType.mult, mybir.AluOpType.mult,
                            )
                        hT = wb.tile([P, 8, P], bf16, tag="hT")
                        nc.sync.dma_start_transpose(hT[:], h_p[:])
                        if use_fp8:
                            hT8 = wb.tile([P, 8, P], f8, tag="hT8")
                            nc.vector.tensor_copy(hT8[:], hT[:])
                        else:
                            hT8 = hT

                    # --- PE: swiglu mms for tile i ---
                    if i < n_tiles:
                        h = wb.tile([P, D], bf16, tag="h")
                        pairs = []
                        for j in range(2):
                            pu = ps_uv.tile([P, H], fp32, tag="uv", name=f"uv_u{j}")
                            pg = ps_uv.tile([P, H], fp32, tag="uv", name=f"uv_g{j}")
                            nk = 4 if use_fp8 else 8
                            for m in range(nk):
                                ms = slice(2 * m, 2 * m + 2) if use_fp8 else m
                                nc.tensor.matmul(
                                    pu[:], lhsT=y2T8[:, ms, :],
                                    rhs=wsw_sb[:, ms, j * H : (j + 1) * H],
                                    start=(m == 0), stop=(m == nk - 1),
                                    perf_mode=DR,
                                )
                                nc.tensor.matmul(
                                    pg[:], lhsT=y2T8[:, ms, :],
                                    rhs=wsw_sb[:, ms, (2 + j) * H : (3 + j) * H],
                                    start=(m == 0), stop=(m == nk - 1),
                                    perf_mode=DR,
                                )
                            pairs.append((pu, pg))

                    # --- PE: out mms for the previous tile ---
                    if prev is not None:
                        op = [
                            ps_o.tile([P, H], fp32, tag="op", name=f"op{n}")
                            for n in range(2)
                        ]
                        nk = 4 if use_fp8 else 8
                        for m in range(nk):
                            ms = slice(2 * m, 2 * m + 2) if use_fp8 else m
                            for n in range(2):
                                nc.tensor.matmul(
                                    op[n][:], lhsT=hT8[:, ms, :],
                                    rhs=wout_sb[:, ms, n * H : (n + 1) * H],
                                    start=(m == 0), stop=(m == nk - 1),
                                    perf_mode=DR,
                                )
                        xt2 = wb.tile([P, D], fp32, tag="xt2")
                        nc.sync.dma_start(xt2[:], x_t[ip])
                        for n in range(2):
                            ns = slice(n * H, (n + 1) * H)
                            # op is 512x (h 32x, w_out 16x)
                            nc.vector.scalar_tensor_tensor(
                                xt2[:, ns], op[n][:], 1.0 / 512.0, xt2[:, ns],
                                mybir.AluOpType.mult, mybir.AluOpType.add,
                            )
                        nc.sync.dma_start(out_t[ip], xt2[:])

                    if i < n_tiles:
                        prev = (pairs, h, i)

    nc.compile()
    return nc


def _host_inputs(x, w_qkv, w_swiglu, w_out, use_fp8=True):
    bf = ml_dtypes.bfloat16
    f8 = ml_dtypes.float8_e4m3fn if use_fp8 else bf
    # weights are ~uniform(+-0.054): scale 16x so the fp8 cast stays out of
    # e4m3's subnormal range (min normal 2^-6).  q/k are rms-normalized so
    # the scale cancels; the v->kv->y2 chain carries it (y2 16x helps its
    # own fp8 cast); phase B undoes 256x/2048x in fused scalar ops.
    wqkvT = np.ascontiguousarray(w_qkv.T * 16.0).astype(f8)
    wswT = np.ascontiguousarray(w_swiglu.T * 16.0).astype(f8)
    woutT = np.ascontiguousarray(w_out.T * 16.0).astype(f8)
    tri = np.triu(np.ones((P, P), np.float32)).astype(bf)
    in_maps = []
    for c in range(N_CORES):
        b, h = c // 2, c % 2
        in_maps.append(
            {
                "x": np.ascontiguousarray(
                    x[b, h * CHUNK : (h + 1) * CHUNK, :]
                ).astype(np.float32),
                "wqkvT": wqkvT,
                "wswT": wswT,
                "woutT": woutT,
                "triT": tri,
                "cmask": np.full((2, 1), float(h), np.float32),
            }
        )
    return in_maps


def kernel(x, w_qkv, w_swiglu, w_out, trace=False):
    from concourse.bass_utils import run_bass_kernel_spmd

    x = np.asarray(x, dtype=np.float32)
    w_qkv = np.asarray(w_qkv, dtype=np.float32)
    w_swiglu = np.asarray(w_swiglu, dtype=np.float32)
    w_out = np.asarray(w_out, dtype=np.float32)

    key = "full"
    if key not in _nc_cache:
        _nc_cache[key] = build_nc(NT_FULL, N_CORES, use_collective=True, use_fp8=USE_FP8)
    nc = _nc_cache[key]

    in_maps = _host_inputs(x, w_qkv, w_swiglu, w_out, use_fp8=USE_FP8)
    res = run_bass_kernel_spmd(
        nc, in_maps, core_ids=list(range(N_CORES)), trace=trace
    )
    out = np.empty((B_FULL, T_FULL, D), np.float32)
    for c in range(N_CORES):
        b, h = c // 2, c % 2
        out[b, h * CHUNK : (h + 1) * CHUNK, :] = res.results[c]["out"]
    kernel.last_result = res
    return out



# revision 45
# speedup vs baseline: 1.1024x; 1.1024x over previous
"""AFT (attention-free transformer) block kernel for 8 Trainium2 NeuronCores.

Sharding: batch b in 0..3 -> core pair (2b, 2b+1); each core handles 4096
contiguous tokens of that batch's 8192-token sequence.  The only cross-core
dependency is the cumsum carry (per-channel totals of w=exp(k_norm) and
kv=w*v over the first half), exchanged with a per-pair AllGather; odd cores
apply the received carry, even cores multiply it by a 0 mask.

Layout: everything is [token=partition, channel=free].  Matmuls run in bf16
(inputs pre-transposed).  The per-128-token cumsum is a lower-triangular
matmul on the PE; the running carry stays fp32 and already broadcast across
partitions: an all-ones matmul yields the tile's column-sum replicated on
every partition, so the carry update is a single vector add per tile.
"""

import sys
import numpy as np
import ml_dtypes

for _p in ("/opt/trn_rl_repo",):
    if _p not in sys.path:
        sys.path.insert(0, _p)

P = 128
D = 1024
N_CORES = 8
B_FULL, T_FULL = 4, 8192
CHUNK = T_FULL // 2          # tokens per core
NT_FULL = CHUNK // P         # 32 tiles per core
RMS_EPS = 1.1920929e-07
AFT_EPS = 1e-6

_nc_cache = {}
USE_FP8 = True

_ACT_TABLES_PATCHED = False


def _restrict_act_tables():
    # Confine activation-table choice to two sets (phase A: ln/exp/square,
    # phase B: sigmoid) so the ACT engine loads each table once instead of
    # thrashing between per-function tables. Ids (dict order) are preserved;
    # emptied sets are merely unchoosable.
    global _ACT_TABLES_PATCHED
    if _ACT_TABLES_PATCHED:
        return
    import concourse.bacc as bacc_mod

    keep = {"natural_log_exp_and_others", "silu_and_others"}
    orig = bacc_mod.get_activation_tables

    def restricted(arch, _orig=orig, _keep=keep):
        return {
            name: (funcs if name in _keep else set())
            for name, funcs in _orig(arch).items()
        }

    bacc_mod.get_activation_tables = restricted
    _ACT_TABLES_PATCHED = True


def build_nc(n_tiles=NT_FULL, num_devices=N_CORES, use_collective=True, use_fp8=True):
    import concourse.mybir as mybir
    import concourse.tile as tile
    from concourse import bacc

    AF = mybir.ActivationFunctionType
    fp32 = mybir.dt.float32
    bf16 = mybir.dt.bfloat16
    f8 = mybir.dt.float8e4 if use_fp8 else mybir.dt.bfloat16
    DR = mybir.MatmulPerfMode.DoubleRow if use_fp8 else None
    chunk = n_tiles * P

    _restrict_act_tables()
    nc = bacc.Bacc(
        "TRN2",
        target_bir_lowering=False,
        debug=False,
        enable_asserts=False,
        num_devices=num_devices,
    )

    x_d = nc.dram_tensor("x", [chunk, D], fp32, kind="ExternalInput")
    wqkv_d = nc.dram_tensor("wqkvT", [D, 3 * D], f8, kind="ExternalInput")
    wsw_d = nc.dram_tensor("wswT", [D, 2 * D], f8, kind="ExternalInput")
    wout_d = nc.dram_tensor("woutT", [D, D], f8, kind="ExternalInput")
    tri_d = nc.dram_tensor("triT", [P, P], bf16, kind="ExternalInput")
    mask_d = nc.dram_tensor("cmask", [2, 1], fp32, kind="ExternalInput")
    out_d = nc.dram_tensor("out", [chunk, D], fp32, kind="ExternalOutput")

    x_t = x_d.ap().rearrange("(n p) d -> n p d", p=P)
    out_t = out_d.ap().rearrange("(n p) d -> n p d", p=P)

    H = D // 2  # 512, matmul free-dim chunk

    with tile.TileContext(nc) as tc:
        with (
            tc.tile_pool(name="consts", bufs=1) as consts,
            tc.tile_pool(name="wbl", bufs=3) as wbl,
            tc.tile_pool(name="dram", bufs=1, space="DRAM") as dram,
        ):
            # ---- persistent constants in SBUF ----
            tri_sb = consts.tile([P, P], bf16)
            nc.sync.dma_start(tri_sb[:], tri_d.ap())
            ones_col = consts.tile([1, P], bf16)
            nc.any.memset(ones_col[:], 1.0)
            mask_sb = consts.tile([2, 1], fp32)
            nc.sync.dma_start(mask_sb[:], mask_d.ap())
            eps_sb = consts.tile([P, 1], fp32)
            nc.any.memset(eps_sb[:], RMS_EPS)
            ln2_sb = consts.tile([P, 1], fp32)
            nc.any.memset(ln2_sb[:], float(np.log(2.0)))

            # ---- DRAM scratch for phase A -> B ----
            wcum_dram = dram.tile([n_tiles, P, D], bf16)
            kvcum_dram = dram.tile([n_tiles, P, D], bf16)
            sigq_dram = dram.tile([n_tiles, P, D], bf16)
            cc_in = dram.tile([2, D], fp32)
            cc_out = dram.tile([4, D], fp32)

            # phase-B weights get a dedicated pool that coexists with phase A
            # so their SWDGE loads overlap phase A instead of waiting on a
            # WAR-reused SBUF range at the phase boundary.
            wsw_sb = consts.tile([P, 8, 2 * D], f8)
            wout_sb = consts.tile([P, 8, D], f8)

            # x stash: loaded once as bf16 (SWDGE cast-DMA), reused in
            # phase B for the residual add -- saves the fp32 x re-read
            x_sb = consts.tile([P, n_tiles, D], bf16)

            # =========================== PHASE A ===========================
            with (
                tc.tile_pool(name="ps_qkv", bufs=6, space="PSUM") as ps_qkv,
                tc.tile_pool(name="ps_scan", bufs=2, space="PSUM") as ps_scan,
                tc.tile_pool(name="wka", bufs=3) as wk,
                tc.tile_pool(name="cbp", bufs=3) as cbp,
                tc.tile_pool(name="wqa", bufs=1) as wqa,
            ):
                wq_ap = wqkv_d.ap().rearrange("(ko p) n -> p ko n", p=P)
                wq_sb = wqa.tile([P, 8, 3 * D], f8, name="wq_sb")
                for kk in range(8):
                    nc.gpsimd.dma_start(wq_sb[:, kk, :], wq_ap[:, kk, :])
                wsw_ap = wsw_d.ap().rearrange("(ko p) n -> p ko n", p=P)
                wout_ap = wout_d.ap().rearrange("(ko p) n -> p ko n", p=P)
                bweight_dmas = []
                for kk in range(8):
                    bweight_dmas.append(
                        nc.gpsimd.dma_start(wsw_sb[:, kk, :], wsw_ap[:, kk, :])
                    )
                    bweight_dmas.append(
                        nc.gpsimd.dma_start(wout_sb[:, kk, :], wout_ap[:, kk, :])
                    )

                # inter-tile carry rows: (tile, row) with row 127 of the
                # previous spilled cum tile (an AP slice, no copy); zero
                # rows for tile 0
                cb = {}
                for t in ("w", "kv"):
                    zrow = cbp.tile([1, D], bf16, tag=f"cb_{t}", name=f"cb_{t}")
                    nc.any.memzero(zrow[:])
                    cb[t] = (zrow, 0)

                def scan_emit(w_sb, kv_sb, i):
                    # chunked causal cumsum; the inter-tile carry is row 127
                    # of the previous cum tile, folded into the scan PSUM via
                    # a rank-1 matmul; deferred one tile behind the qkv mms
                    # so the PE never waits on this tile's ACT/DVE chain
                    for t, src, dst, ceng in (
                        ("w", w_sb, wcum_dram, "act"),
                        ("kv", kv_sb, kvcum_dram, "dve"),
                    ):
                        cum = wk.tile([P, D], bf16, tag=f"cum_{t}", name=f"cum_{t}", bufs=3)
                        cbt, cbr = cb[t]
                        for j in range(2):
                            js = slice(j * H, (j + 1) * H)
                            ps = ps_scan.tile(
                                [P, H], fp32, tag="scan", name=f"scan_{t}{j}"
                            )
                            nc.tensor.matmul(
                                ps[:], lhsT=tri_sb[:], rhs=src[:, js],
                                start=True, stop=False,
                            )
                            nc.tensor.matmul(
                                ps[:], lhsT=ones_col[:],
                                rhs=cbt[cbr : cbr + 1, js],
                                start=False, stop=True,
                            )
                            if ceng == "act":
                                nc.scalar.copy(cum[:, js], ps[:])
                            else:
                                nc.vector.tensor_copy(cum[:, js], ps[:])
                        nc.sync.dma_start(dst[i], cum[:])
                        # matmul operands must sit at partition 0/32/64, so
                        # hop row 127 down via a tiny SWDGE sbuf->sbuf DMA
                        nxt = cbp.tile([1, D], bf16, tag=f"cb_{t}", name=f"cbn_{t}")
                        nc.gpsimd.dma_start(nxt[:], cum[127:128, :])
                        cb[t] = (nxt, 0)
                        if i + 1 == n_tiles:
                            row = 0 if t == "w" else 1
                            nc.gpsimd.dma_start(
                                cc_in[row : row + 1, :], cum[127:128, :]
                            )

                prevA = None  # (w_sb, kv_sb, i) awaiting the deferred scan
                for i in range(n_tiles + 1):
                    if i < n_tiles:
                        # x as bf16 into the cross-phase stash.  rms_norm is
                        # scale-invariant per token, so q/k need no
                        # normalized x at all; only v needs the rs scale
                        # (folded into the kv mul below).
                        nc.gpsimd.dma_start(x_sb[:, i, :], x_t[i])
                        xb = x_sb[:, i, :]

                        sq = wk.tile([P, D], bf16, tag="sqscratch", bufs=2)
                        ssq = wk.tile([P, 1], fp32, tag="ssq")
                        nc.scalar.activation(sq[:], xb, AF.Square, accum_out=ssq[:])
                        # rsqrt via exp(-0.5*ln(mean+eps)): ln/exp table
                        lms = wk.tile([P, 1], fp32, tag="lms")
                        nc.scalar.activation(
                            lms[:], ssq[:], AF.Ln, scale=1.0 / D, bias=eps_sb[:]
                        )
                        rs = wk.tile([P, 1], fp32, tag="rs")
                        nc.scalar.activation(rs[:], lms[:], AF.Exp, scale=-0.5)

                        # transpose x for matmul lhsT (single xbar DMA)
                        xnT = wk.tile([P, 8, P], bf16, tag="xnT", bufs=2)
                        nc.sync.dma_start_transpose(xnT[:], xb)
                        if use_fp8:
                            xnT8 = wk.tile([P, 8, P], f8, tag="xnT8", bufs=2)
                            nc.gpsimd.tensor_copy(xnT8[:], xnT[:])
                        else:
                            xnT8 = xnT

                        # qkv chunk-pair matmul: chunks (c0, c0+1) of 6x512
                        def mm_pair(c0, xnT8=xnT8):
                            pair = [
                                ps_qkv.tile(
                                    [P, H], fp32, tag="qkv", name=f"qkv{c0}_{c}"
                                )
                                for c in range(2)
                            ]
                            nk = 4 if use_fp8 else 8
                            for m in range(nk):
                                ms = slice(2 * m, 2 * m + 2) if use_fp8 else m
                                for ci in range(2):
                                    nc.tensor.matmul(
                                        pair[ci][:],
                                        lhsT=xnT8[:, ms, :],
                                        rhs=wq_sb[
                                            :, ms, (c0 + ci) * H : (c0 + ci + 1) * H
                                        ],
                                        start=(m == 0),
                                        stop=(m == nk - 1),
                                        perf_mode=DR,
                                    )
                            return pair

                        def rms_scale(pair, nm):
                            sq2 = wk.tile(
                                [P, H], bf16, tag="sqscratch",
                                name=f"sq2_{nm}", bufs=2,
                            )
                            pa = wk.tile([P, 1], fp32, tag=f"pa_{nm}", name=f"pa_{nm}")
                            pb = wk.tile([P, 1], fp32, tag=f"pb_{nm}", name=f"pb_{nm}")
                            nc.scalar.activation(
                                sq2[:], pair[0][:], AF.Square, accum_out=pa[:]
                            )
                            nc.scalar.activation(
                                sq2[:], pair[1][:], AF.Square, accum_out=pb[:]
                            )
                            st = wk.tile([P, 1], fp32, tag=f"st_{nm}", name=f"st_{nm}")
                            nc.vector.tensor_add(st[:], pa[:], pb[:])
                            nc.scalar.activation(
                                st[:], st[:], AF.Ln, scale=1.0 / D, bias=eps_sb[:]
                            )
                            rr = wk.tile([P, 1], fp32, tag=f"rr_{nm}", name=f"rr_{nm}")
                            nc.scalar.activation(rr[:], st[:], AF.Exp, scale=-0.5)
                            return rr

                        # k chunks -> w = exp(rms(k))
                        kp = mm_pair(2)
                        rsk = rms_scale(kp, "k")
                        w_sb = wk.tile([P, D], bf16, tag="w_sb", bufs=3)
                        # w' = 2*exp(rms(k)) = exp(rms(k) + ln2): the
                        # doubling makes 1/(w'cum+2eps) = 0.5/(wcum+eps),
                        # absorbing the 0.5 of the tanh sigmoid for free
                        for j in range(2):
                            js = slice(j * H, (j + 1) * H)
                            nc.scalar.activation(
                                w_sb[:, js], kp[j][:], AF.Exp, scale=rsk[:],
                                bias=ln2_sb[:],
                            )

                        # q chunks -> rms(q) spilled (tanh sigmoid in B)
                        qp = mm_pair(0)
                        rsq = rms_scale(qp, "q")
                        sigq = wk.tile([P, D], bf16, tag="sigq", bufs=2)
                        for j in range(2):
                            js = slice(j * H, (j + 1) * H)
                            nc.vector.tensor_scalar_mul(
                                sigq[:, js], qp[j][:], rsq[:]
                            )
                        nc.sync.dma_start(sigq_dram[i], sigq[:])

                        # v chunks -> kv = w * (rs * v): x rms scale rides here
                        vp = mm_pair(4)
                        kv_sb = wk.tile([P, D], bf16, tag="kv_sb", bufs=3)
                        for j in range(2):
                            js = slice(j * H, (j + 1) * H)
                            nc.vector.scalar_tensor_tensor(
                                kv_sb[:, js], vp[j][:], rs[:], w_sb[:, js],
                                mybir.AluOpType.mult, mybir.AluOpType.mult,
                            )

                    if prevA is not None:
                        scan_emit(*prevA)
                    prevA = (w_sb, kv_sb, i) if i < n_tiles else None

            # ======================= carry exchange ========================
            import concourse.mybir as _mybir

            gath = consts.tile([2, D], fp32)
            if use_collective:
                nc.gpsimd.collective_compute(
                    "AllGather",
                    _mybir.AluOpType.bypass,
                    replica_groups=[[2 * p, 2 * p + 1] for p in range(num_devices // 2)],
                    ins=[cc_in[:].opt()],
                    outs=[cc_out[:].opt()],
                    cc_dim="Partition",
                )
                nc.sync.dma_start(gath[:], cc_out[0:2, :])
            else:
                nc.any.memzero(gath[:])

            # mask, downcast to bf16, and fold the denominator epsilon into
            # the w-carry row (2x: spilled w-cumsums carry the exp doubling)
            rows16 = consts.tile([2, D], bf16)
            nc.vector.tensor_scalar_mul(rows16[:], gath[:], mask_sb[:])
            nc.vector.tensor_scalar_add(
                rows16[0:1, :], rows16[0:1, :], 2 * AFT_EPS
            )
            row1b = consts.tile([1, D], bf16)
            nc.sync.dma_start(row1b[:], rows16[1:2, :])
            cwb = consts.tile([P, D], bf16)
            ckb = consts.tile([P, D], bf16)
            nc.gpsimd.partition_broadcast(cwb[:], rows16[0:1, :])
            nc.gpsimd.partition_broadcast(ckb[:], row1b[:])

            # =========================== PHASE B ===========================
            # Software-pipelined: the y chain for tile i+1 is emitted while
            # the PE consumes tile i's y2T8 (made last iteration), so the PE
            # only ever depends on results that are a full tile old.
            with (
                tc.tile_pool(name="ps_uv", bufs=5, space="PSUM") as ps_uv,
                tc.tile_pool(name="ps_o", bufs=3, space="PSUM") as ps_o,
                tc.tile_pool(name="wkb", bufs=2) as wb,
            ):
                loads = {}

                def emit_loads(i):
                    wc = wbl.tile([P, D], bf16, tag="wc", bufs=3)
                    nc.sync.dma_start(wc[:], wcum_dram[i])
                    kc = wbl.tile([P, D], bf16, tag="kc", bufs=3)
                    nc.sync.dma_start(kc[:], kvcum_dram[i])
                    sgq = wbl.tile([P, D], bf16, tag="sgq", bufs=3)
                    nc.sync.dma_start(sgq[:], sigq_dram[i])
                    loads[i] = (wc, kc, sgq)

                def ychain_head(i):
                    # y2 = sigmoid(q)*kvc/wc with sigmoid(q) =
                    # 0.5*(tanh(q/2)+1); the 0.5 hides in the w doubling
                    wc, kc, sgq = loads.pop(i)
                    sig = wb.tile([P, D], bf16, tag="sig")
                    nc.scalar.activation(sig[:], sgq[:], AF.Tanh, scale=0.5)
                    twc = wb.tile([P, D], bf16, tag="twc")
                    nc.gpsimd.tensor_add(twc[:], wc[:], cwb[:])
                    rec = wb.tile([P, D], bf16, tag="rec")
                    with nc.allow_low_precision(reason="y denominators are bf16 anyway"):
                        nc.vector.reciprocal(rec[:], twc[:])
                    tkc = wb.tile([P, D], bf16, tag="tkc")
                    nc.gpsimd.tensor_add(tkc[:], kc[:], ckb[:])
                    yt = wb.tile([P, D], bf16, tag="yt")
                    nc.vector.tensor_mul(yt[:], tkc[:], rec[:])
                    y2 = wb.tile([P, D], bf16, tag="y2")
                    nc.vector.scalar_tensor_tensor(
                        y2[:], sig[:], 1.0, yt[:],
                        mybir.AluOpType.add, mybir.AluOpType.mult,
                    )
                    y2T = wb.tile([P, 8, P], bf16, tag="y2T", bufs=3)
                    nc.sync.dma_start_transpose(y2T[:], y2[:])
                    return y2T

                def ychain_tail(y2T):
                    if not use_fp8:
                        return y2T
                    y2T8 = wb.tile([P, 8, P], f8, tag="y2T8", bufs=3)
                    nc.scalar.copy(y2T8[:], y2T[:])
                    return y2T8

                if n_tiles > 0:
                    emit_loads(0)
                    if n_tiles > 1:
                        emit_loads(1)
                    cur = ychain_tail(ychain_head(0))
                prev = None  # (pairs, h, i) consumed one iteration later
                for i in range(n_tiles + 1):
                    if i + 2 < n_tiles:
                        emit_loads(i + 2)
                    y2T_next = ychain_head(i + 1) if i + 1 < n_tiles else None

                    # --- PE: swiglu mms for tile i (inputs one tile old) ---
                    if i < n_tiles:
                        h = wb.tile([P, D], bf16, tag="h", bufs=3)
                        pairs = []
                        for j in range(2):
                            pu = ps_uv.tile([P, H], fp32, tag="uv", name=f"uv_u{j}")
                            pg = ps_uv.tile([P, H], fp32, tag="uv", name=f"uv_g{j}")
                            nk = 4 if use_fp8 else 8
                            for m in range(nk):
                                ms = slice(2 * m, 2 * m + 2) if use_fp8 else m
                                nc.tensor.matmul(
                                    pu[:], lhsT=cur[:, ms, :],
                                    rhs=wsw_sb[:, ms, j * H : (j + 1) * H],
                                    start=(m == 0), stop=(m == nk - 1),
                                    perf_mode=DR,
                                )
                                nc.tensor.matmul(
                                    pg[:], lhsT=cur[:, ms, :],
                                    rhs=wsw_sb[:, ms, (2 + j) * H : (3 + j) * H],
                                    start=(m == 0), stop=(m == nk - 1),
                                    perf_mode=DR,
                                )
                            pairs.append((pu, pg))

                    # --- previous tile: silu/h, out mms, residual, store ---
                    if prev is not None:
                        pairs_p, h_p, ip = prev
                        for j, (pu, pg) in enumerate(pairs_p):
                            js = slice(j * H, (j + 1) * H)
                            # weights are host-scaled 16x, y2 carries 32x
                            # (16x weights + the 2x w-doubling), so pg/pu
                            # are 512x; h is kept at 32x so its fp8 cast
                            # stays clear of e4m3 subnormals yet under 448
                            sl = wb.tile([P, H], bf16, tag="sl", name=f"sl{j}", bufs=3)
                            nc.scalar.activation(
                                sl[:], pg[:], AF.Silu, scale=1.0 / 512.0
                            )
                            nc.vector.scalar_tensor_tensor(
                                h_p[:, js], pu[:], 1.0 / 16.0, sl[:],
                                mybir.AluOpType.mult, mybir.AluOpType.mult,
                            )
                        hT = wb.tile([P, 8, P], bf16, tag="hT")
                        nc.sync.dma_start_transpose(hT[:], h_p[:])
                        if use_fp8:
                            hT8 = wb.tile([P, 8, P], f8, tag="hT8")
                            nc.gpsimd.tensor_copy(hT8[:], hT[:])
                        else:
                            hT8 = hT
                        op = [
                            ps_o.tile([P, H], fp32, tag="op", name=f"op{n}")
                            for n in range(2)
                        ]
                        nk = 4 if use_fp8 else 8
                        for m in range(nk):
                            ms = slice(2 * m, 2 * m + 2) if use_fp8 else m
                            for n in range(2):
                                nc.tensor.matmul(
                                    op[n][:], lhsT=hT8[:, ms, :],
                                    rhs=wout_sb[:, ms, n * H : (n + 1) * H],
                                    start=(m == 0), stop=(m == nk - 1),
                                    perf_mode=DR,
                                )
                        xt2 = wb.tile([P, D], fp32, tag="xt2")
                        for n in range(2):
                            ns = slice(n * H, (n + 1) * H)
                            # op is 512x (h 32x, w_out 16x); residual comes
                            # from the bf16 x stash, no re-read
                            nc.vector.scalar_tensor_tensor(
                                xt2[:, ns], op[n][:], 1.0 / 512.0,
                                x_sb[:, ip, ns],
                                mybir.AluOpType.mult, mybir.AluOpType.add,
                            )
                        nc.sync.dma_start(out_t[ip], xt2[:])

                    prev = (pairs, h, i) if i < n_tiles else None
                    cur = ychain_tail(y2T_next) if y2T_next is not None else None

    nc.compile()
    return nc


def _host_inputs(x, w_qkv, w_swiglu, w_out, use_fp8=True):
    bf = ml_dtypes.bfloat16
    f8 = ml_dtypes.float8_e4m3fn if use_fp8 else bf
    # weights are ~uniform(+-0.054): scale 16x so the fp8 cast stays out of
    # e4m3's subnormal range (min normal 2^-6).  q/k are rms-normalized so
    # the scale cancels; the v->kv->y2 chain carries it (y2 16x helps its
    # own fp8 cast); phase B undoes 256x/2048x in fused scalar ops.
    wqkvT = np.ascontiguousarray(w_qkv.T * 16.0).astype(f8)
    wswT = np.ascontiguousarray(w_swiglu.T * 16.0).astype(f8)
    woutT = np.ascontiguousarray(w_out.T * 16.0).astype(f8)
    tri = np.triu(np.ones((P, P), np.float32)).astype(bf)
    in_maps = []
    for c in range(N_CORES):
        b, h = c // 2, c % 2
        in_maps.append(
            {
                "x": np.ascontiguousarray(
                    x[b, h * CHUNK : (h + 1) * CHUNK, :]
                ).astype(np.float32),
                "wqkvT": wqkvT,
                "wswT": wswT,
                "woutT": woutT,
                "triT": tri,
                "cmask": np.full((2, 1), float(h), np.float32),
            }
        )
    return in_maps


def kernel(x, w_qkv, w_swiglu, w_out, trace=False):
    from concourse.bass_utils import run_bass_kernel_spmd

    x = np.asarray(x, dtype=np.float32)
    w_qkv = np.asarray(w_qkv, dtype=np.float32)
    w_swiglu = np.asarray(w_swiglu, dtype=np.float32)
    w_out = np.asarray(w_out, dtype=np.float32)

    key = "full"
    if key not in _nc_cache:
        _nc_cache[key] = build_nc(NT_FULL, N_CORES, use_collective=True, use_fp8=USE_FP8)
    nc = _nc_cache[key]

    in_maps = _host_inputs(x, w_qkv, w_swiglu, w_out, use_fp8=USE_FP8)
    res = run_bass_kernel_spmd(
        nc, in_maps, core_ids=list(range(N_CORES)), trace=trace
    )
    out = np.empty((B_FULL, T_FULL, D), np.float32)
    for c in range(N_CORES):
        b, h = c // 2, c % 2
        out[b, h * CHUNK : (h + 1) * CHUNK, :] = res.results[c]["out"]
    kernel.last_result = res
    return out



# revision 63
# speedup vs baseline: 1.2988x; 1.1781x over previous
"""AFT (attention-free transformer) block kernel for 8 Trainium2 NeuronCores.

Sharding: batch b in 0..3 -> core pair (2b, 2b+1); each core handles 4096
contiguous tokens of that batch's 8192-token sequence.  The only cross-core
dependency is the cumsum carry (per-channel totals of w=exp(k_norm) and
kv=w*v over the first half), exchanged with a per-pair AllGather; odd cores
apply the received carry, even cores multiply it by a 0 mask.

Layout: everything is [token=partition, channel=free].  Matmuls run in bf16
(inputs pre-transposed).  The per-128-token cumsum is a lower-triangular
matmul on the PE; the running carry stays fp32 and already broadcast across
partitions: an all-ones matmul yields the tile's column-sum replicated on
every partition, so the carry update is a single vector add per tile.
"""

import sys
import numpy as np
import ml_dtypes

for _p in ("/opt/trn_rl_repo",):
    if _p not in sys.path:
        sys.path.insert(0, _p)

P = 128
D = 1024
N_CORES = 8
B_FULL, T_FULL = 4, 8192
CHUNK = T_FULL // 2          # tokens per core
NT_FULL = CHUNK // P         # 32 tiles per core
RMS_EPS = 1.1920929e-07
AFT_EPS = 1e-6

_nc_cache = {}
USE_FP8 = True

_ACT_TABLES_PATCHED = False


def _restrict_act_tables():
    # Confine activation-table choice to two sets (phase A: ln/exp/square,
    # phase B: sigmoid) so the ACT engine loads each table once instead of
    # thrashing between per-function tables. Ids (dict order) are preserved;
    # emptied sets are merely unchoosable.
    global _ACT_TABLES_PATCHED
    if _ACT_TABLES_PATCHED:
        return
    import concourse.bacc as bacc_mod

    keep = {"natural_log_exp_and_others", "silu_and_others"}
    orig = bacc_mod.get_activation_tables

    def restricted(arch, _orig=orig, _keep=keep):
        return {
            name: (funcs if name in _keep else set())
            for name, funcs in _orig(arch).items()
        }

    bacc_mod.get_activation_tables = restricted
    _ACT_TABLES_PATCHED = True


def build_nc(n_tiles=NT_FULL, num_devices=N_CORES, use_collective=True, use_fp8=True):
    import concourse.mybir as mybir
    import concourse.tile as tile
    from concourse import bacc

    AF = mybir.ActivationFunctionType
    fp32 = mybir.dt.float32
    bf16 = mybir.dt.bfloat16
    f8 = mybir.dt.float8e4 if use_fp8 else mybir.dt.bfloat16
    DR = mybir.MatmulPerfMode.DoubleRow if use_fp8 else None
    chunk = n_tiles * P

    _restrict_act_tables()
    nc = bacc.Bacc(
        "TRN2",
        target_bir_lowering=False,
        debug=False,
        enable_asserts=False,
        num_devices=num_devices,
    )

    x_d = nc.dram_tensor("x", [chunk, D], fp32, kind="ExternalInput")
    wqkv_d = nc.dram_tensor("wqkvT", [D, 3 * D], f8, kind="ExternalInput")
    wsw_d = nc.dram_tensor("wswT", [D, 2 * D], f8, kind="ExternalInput")
    wout_d = nc.dram_tensor("woutT", [D, D], f8, kind="ExternalInput")
    tri_d = nc.dram_tensor("triT", [P, P], bf16, kind="ExternalInput")
    mask_d = nc.dram_tensor("cmask", [2, 1], fp32, kind="ExternalInput")
    out_d = nc.dram_tensor("out", [chunk, D], fp32, kind="ExternalOutput")

    x_t = x_d.ap().rearrange("(n p) d -> n p d", p=P)
    out_t = out_d.ap().rearrange("(n p) d -> n p d", p=P)

    H = D // 2  # 512, matmul free-dim chunk

    with tile.TileContext(nc) as tc:
        with (
            tc.tile_pool(name="consts", bufs=1) as consts,
            tc.tile_pool(name="wbl", bufs=3) as wbl,
            tc.tile_pool(name="dram", bufs=1, space="DRAM") as dram,
        ):
            # ---- persistent constants in SBUF ----
            tri_sb = consts.tile([P, P], bf16)
            nc.sync.dma_start(tri_sb[:], tri_d.ap())
            ones_col = consts.tile([1, P], bf16)
            nc.any.memset(ones_col[:], 1.0)
            ones_sb = consts.tile([P, P], bf16)
            nc.any.memset(ones_sb[:], 1.0)
            mask_sb = consts.tile([2, 1], fp32)
            nc.sync.dma_start(mask_sb[:], mask_d.ap())
            eps_sb = consts.tile([P, 1], fp32)
            nc.any.memset(eps_sb[:], RMS_EPS)
            ln2_sb = consts.tile([P, 1], fp32)
            nc.any.memset(ln2_sb[:], float(np.log(2.0)))

            # ---- DRAM scratch for phase A -> B ----
            wcum_dram = dram.tile([n_tiles, P, D], bf16)
            kvcum_dram = dram.tile([n_tiles, P, D], bf16)
            sigq_dram = dram.tile([n_tiles, P, D], bf16)
            cc_in = dram.tile([2, D], fp32)
            cc_out = dram.tile([4, D], fp32)

            # phase-B weights get a dedicated pool that coexists with phase A
            # so their SWDGE loads overlap phase A instead of waiting on a
            # WAR-reused SBUF range at the phase boundary.
            wsw_sb = consts.tile([P, 8, 2 * D], f8)
            wout_sb = consts.tile([P, 8, D], f8)

            # x stash: loaded once as bf16 (SWDGE cast-DMA), reused in
            # phase B for the residual add -- saves the fp32 x re-read
            x_sb = consts.tile([P, n_tiles, D], bf16)

            # =========================== PHASE A ===========================
            with (
                tc.tile_pool(name="ps_qkv", bufs=6, space="PSUM") as ps_qkv,
                tc.tile_pool(name="ps_scan", bufs=2, space="PSUM") as ps_scan,
                tc.tile_pool(name="wka", bufs=3) as wk,
                tc.tile_pool(name="cbp", bufs=3) as cbp,
                tc.tile_pool(name="wqa", bufs=1) as wqa,
            ):
                wq_ap = wqkv_d.ap().rearrange("(ko p) n -> p ko n", p=P)
                wq_sb = wqa.tile([P, 8, 3 * D], f8, name="wq_sb")
                for kk in range(8):
                    nc.gpsimd.dma_start(wq_sb[:, kk, :], wq_ap[:, kk, :])
                wsw_ap = wsw_d.ap().rearrange("(ko p) n -> p ko n", p=P)
                wout_ap = wout_d.ap().rearrange("(ko p) n -> p ko n", p=P)
                bweight_dmas = []
                for kk in range(8):
                    bweight_dmas.append(
                        nc.gpsimd.dma_start(wsw_sb[:, kk, :], wsw_ap[:, kk, :])
                    )
                    bweight_dmas.append(
                        nc.gpsimd.dma_start(wout_sb[:, kk, :], wout_ap[:, kk, :])
                    )

                # inter-tile carry rows: (tile, row) with row 127 of the
                # previous spilled cum tile (an AP slice, no copy); zero
                # rows for tile 0
                cb = {}
                for t in ("w", "kv"):
                    zrow = cbp.tile([1, D], bf16, tag=f"cb_{t}", name=f"cb_{t}")
                    nc.any.memzero(zrow[:])
                    cb[t] = (zrow, 0)

                def scan_pair(pa, pb):
                    # chunked causal cumsum over a PAIR of tiles per carry
                    # hop: tile b's PSUM gets tri@b + allones@a + rank1
                    # (carry), so the serial carry link (spill-b -> row hop
                    # -> next rank1) runs once per two tiles
                    w_a, kv_a, ia = pa
                    w_b, kv_b, ib = pb
                    for t, src_a, src_b, dst in (
                        ("w", w_a, w_b, wcum_dram),
                        ("kv", kv_a, kv_b, kvcum_dram),
                    ):
                        cbt, cbr = cb[t]
                        cum_a = wk.tile(
                            [P, D], bf16, tag=f"cum_{t}a", name=f"cum_{t}a", bufs=2
                        )
                        cum_b = wk.tile(
                            [P, D], bf16, tag=f"cum_{t}b", name=f"cum_{t}b", bufs=2
                        )
                        for j in range(2):
                            js = slice(j * H, (j + 1) * H)
                            ps = ps_scan.tile(
                                [P, H], fp32, tag="scan", name=f"scan_{t}b{j}"
                            )
                            nc.tensor.matmul(
                                ps[:], lhsT=tri_sb[:], rhs=src_b[:, js],
                                start=True, stop=False,
                            )
                            nc.tensor.matmul(
                                ps[:], lhsT=ones_sb[:], rhs=src_a[:, js],
                                start=False, stop=False,
                            )
                            nc.tensor.matmul(
                                ps[:], lhsT=ones_col[:],
                                rhs=cbt[cbr : cbr + 1, js],
                                start=False, stop=True,
                            )
                            nc.vector.tensor_copy(cum_b[:, js], ps[:])
                        # the b-side row hop is the carry link: emit first
                        nxt = cbp.tile([1, D], bf16, tag=f"cb_{t}", name=f"cbn_{t}")
                        nc.gpsimd.dma_start(nxt[:], cum_b[127:128, :])
                        for j in range(2):
                            js = slice(j * H, (j + 1) * H)
                            ps = ps_scan.tile(
                                [P, H], fp32, tag="scan", name=f"scan_{t}a{j}"
                            )
                            nc.tensor.matmul(
                                ps[:], lhsT=tri_sb[:], rhs=src_a[:, js],
                                start=True, stop=False,
                            )
                            nc.tensor.matmul(
                                ps[:], lhsT=ones_col[:],
                                rhs=cbt[cbr : cbr + 1, js],
                                start=False, stop=True,
                            )
                            nc.vector.tensor_copy(cum_a[:, js], ps[:])
                        cb[t] = (nxt, 0)
                        nc.sync.dma_start(dst[ia], cum_a[:])
                        nc.sync.dma_start(dst[ib], cum_b[:])
                        if ib + 1 == n_tiles:
                            row = 0 if t == "w" else 1
                            nc.gpsimd.dma_start(
                                cc_in[row : row + 1, :], cum_b[127:128, :]
                            )

                def xload(i):
                    # x as bf16 into the cross-phase stash (SWDGE cast-DMA),
                    # prefetched two tiles ahead of its consumers
                    nc.gpsimd.dma_start(x_sb[:, i, :], x_t[i])

                def rms_x(i):
                    # rms_norm is scale-invariant per token, so q/k need no
                    # normalized x at all; only v needs rs (in the kv mul)
                    xb = x_sb[:, i, :]
                    sq = wk.tile([P, D], bf16, tag="sqscratch", bufs=2)
                    ssq = wk.tile([P, 1], fp32, tag="ssq", bufs=3)
                    nc.scalar.activation(sq[:], xb, AF.Square, accum_out=ssq[:])
                    # rsqrt via exp(-0.5*ln(mean+eps)): ln/exp table
                    lms = wk.tile([P, 1], fp32, tag="lms", bufs=3)
                    nc.scalar.activation(
                        lms[:], ssq[:], AF.Ln, scale=1.0 / D, bias=eps_sb[:]
                    )
                    rs = wk.tile([P, 1], fp32, tag="rs", bufs=3)
                    nc.scalar.activation(rs[:], lms[:], AF.Exp, scale=-0.5)
                    return rs

                def make_xT8(i):
                    # transpose x for matmul lhsT (single xbar DMA), one
                    # tile ahead so the qkv mms never wait on it
                    xnT = wk.tile([P, 8, P], bf16, tag="xnT", bufs=3)
                    nc.sync.dma_start_transpose(xnT[:], x_sb[:, i, :])
                    if not use_fp8:
                        return xnT
                    xnT8 = wk.tile([P, 8, P], f8, tag="xnT8", bufs=3)
                    nc.gpsimd.tensor_copy(xnT8[:], xnT[:])
                    return xnT8

                if n_tiles > 0:
                    xload(0)
                    if n_tiles > 1:
                        xload(1)
                    xT8_cur = make_xT8(0)
                    rs_cur = rms_x(0)
                pendA = []  # (w_sb, kv_sb, i) awaiting the deferred pair scan
                for i in range(n_tiles + 1):
                    if i + 2 < n_tiles:
                        xload(i + 2)
                    if i + 1 < n_tiles:
                        xT8_next = make_xT8(i + 1)
                        rs_next = rms_x(i + 1)

                    # deferred pair scan first: its inputs are 1-2 tiles
                    # old, so the PE starts the iteration without waiting
                    if len(pendA) >= 2:
                        scan_pair(pendA[0], pendA[1])
                        del pendA[:2]

                    if i < n_tiles:
                        xnT8 = xT8_cur
                        rs = rs_cur

                        # qkv chunk-pair matmul: chunks (c0, c0+1) of 6x512
                        def mm_pair(c0, xnT8=xnT8):
                            pair = [
                                ps_qkv.tile(
                                    [P, H], fp32, tag="qkv", name=f"qkv{c0}_{c}"
                                )
                                for c in range(2)
                            ]
                            nk = 4 if use_fp8 else 8
                            for m in range(nk):
                                ms = slice(2 * m, 2 * m + 2) if use_fp8 else m
                                for ci in range(2):
                                    nc.tensor.matmul(
                                        pair[ci][:],
                                        lhsT=xnT8[:, ms, :],
                                        rhs=wq_sb[
                                            :, ms, (c0 + ci) * H : (c0 + ci + 1) * H
                                        ],
                                        start=(m == 0),
                                        stop=(m == nk - 1),
                                        perf_mode=DR,
                                    )
                            return pair

                        def rms_scale(pair, nm):
                            sq2 = wk.tile(
                                [P, H], bf16, tag="sqscratch",
                                name=f"sq2_{nm}", bufs=2,
                            )
                            pa = wk.tile([P, 1], fp32, tag=f"pa_{nm}", name=f"pa_{nm}")
                            pb = wk.tile([P, 1], fp32, tag=f"pb_{nm}", name=f"pb_{nm}")
                            nc.scalar.activation(
                                sq2[:], pair[0][:], AF.Square, accum_out=pa[:]
                            )
                            nc.scalar.activation(
                                sq2[:], pair[1][:], AF.Square, accum_out=pb[:]
                            )
                            st = wk.tile([P, 1], fp32, tag=f"st_{nm}", name=f"st_{nm}")
                            nc.vector.tensor_add(st[:], pa[:], pb[:])
                            nc.scalar.activation(
                                st[:], st[:], AF.Ln, scale=1.0 / D, bias=eps_sb[:]
                            )
                            rr = wk.tile([P, 1], fp32, tag=f"rr_{nm}", name=f"rr_{nm}")
                            nc.scalar.activation(rr[:], st[:], AF.Exp, scale=-0.5)
                            return rr

                        # k chunks -> w = exp(rms(k))
                        kp = mm_pair(2)
                        rsk = rms_scale(kp, "k")
                        w_sb = wk.tile([P, D], bf16, tag="w_sb", bufs=3)
                        # w' = 2*exp(rms(k)) = exp(rms(k) + ln2): the
                        # doubling makes 1/(w'cum+2eps) = 0.5/(wcum+eps),
                        # absorbing the 0.5 of the tanh sigmoid for free
                        for j in range(2):
                            js = slice(j * H, (j + 1) * H)
                            nc.scalar.activation(
                                w_sb[:, js], kp[j][:], AF.Exp, scale=rsk[:],
                                bias=ln2_sb[:],
                            )

                        # q chunks -> rms(q) spilled (tanh sigmoid in B)
                        qp = mm_pair(0)
                        rsq = rms_scale(qp, "q")
                        sigq = wk.tile([P, D], bf16, tag="sigq", bufs=2)
                        for j in range(2):
                            js = slice(j * H, (j + 1) * H)
                            nc.vector.tensor_scalar_mul(
                                sigq[:, js], qp[j][:], rsq[:]
                            )
                        nc.sync.dma_start(sigq_dram[i], sigq[:])

                        # v chunks -> kv = w * (rs * v): x rms scale rides here
                        vp = mm_pair(4)
                        kv_sb = wk.tile([P, D], bf16, tag="kv_sb", bufs=3)
                        for j in range(2):
                            js = slice(j * H, (j + 1) * H)
                            nc.vector.scalar_tensor_tensor(
                                kv_sb[:, js], vp[j][:], rs[:], w_sb[:, js],
                                mybir.AluOpType.mult, mybir.AluOpType.mult,
                            )

                    if i < n_tiles:
                        pendA.append((w_sb, kv_sb, i))
                    if i + 1 < n_tiles:
                        xT8_cur, rs_cur = xT8_next, rs_next

            # ======================= carry exchange ========================
            import concourse.mybir as _mybir

            gath = consts.tile([2, D], fp32)
            if use_collective:
                nc.gpsimd.collective_compute(
                    "AllGather",
                    _mybir.AluOpType.bypass,
                    replica_groups=[[2 * p, 2 * p + 1] for p in range(num_devices // 2)],
                    ins=[cc_in[:].opt()],
                    outs=[cc_out[:].opt()],
                    cc_dim="Partition",
                )
                nc.sync.dma_start(gath[:], cc_out[0:2, :])
            else:
                nc.any.memzero(gath[:])

            # mask, downcast to bf16, and fold the denominator epsilon into
            # the w-carry row (2x: spilled w-cumsums carry the exp doubling)
            rows16 = consts.tile([2, D], bf16)
            nc.vector.tensor_scalar_mul(rows16[:], gath[:], mask_sb[:])
            nc.vector.tensor_scalar_add(
                rows16[0:1, :], rows16[0:1, :], 2 * AFT_EPS
            )
            row1b = consts.tile([1, D], bf16)
            nc.sync.dma_start(row1b[:], rows16[1:2, :])
            cwb = consts.tile([P, D], bf16)
            ckb = consts.tile([P, D], bf16)
            nc.gpsimd.partition_broadcast(cwb[:], rows16[0:1, :])
            nc.gpsimd.partition_broadcast(ckb[:], row1b[:])

            # =========================== PHASE B ===========================
            # Software-pipelined: the y chain for tile i+1 is emitted while
            # the PE consumes tile i's y2T8 (made last iteration), so the PE
            # only ever depends on results that are a full tile old.
            with (
                tc.tile_pool(name="ps_uv", bufs=5, space="PSUM") as ps_uv,
                tc.tile_pool(name="ps_o", bufs=3, space="PSUM") as ps_o,
                tc.tile_pool(name="wkb", bufs=2) as wb,
            ):
                loads = {}

                def emit_loads(i):
                    wc = wbl.tile([P, D], bf16, tag="wc", bufs=3)
                    nc.sync.dma_start(wc[:], wcum_dram[i])
                    kc = wbl.tile([P, D], bf16, tag="kc", bufs=3)
                    nc.sync.dma_start(kc[:], kvcum_dram[i])
                    sgq = wbl.tile([P, D], bf16, tag="sgq", bufs=3)
                    nc.sync.dma_start(sgq[:], sigq_dram[i])
                    loads[i] = (wc, kc, sgq)

                pres = {}
                mids = {}
                tails = {}

                def ychain_pre(i):
                    # Pool-side carry adds; these only need the DMA loads,
                    # so they go to the front of the Pool queue
                    wc, kc, sgq = loads.pop(i)
                    twc = wb.tile([P, D], bf16, tag="twc", bufs=3)
                    nc.gpsimd.tensor_add(twc[:], wc[:], cwb[:])
                    tkc = wb.tile([P, D], bf16, tag="tkc", bufs=3)
                    nc.gpsimd.tensor_add(tkc[:], kc[:], ckb[:])
                    pres[i] = (twc, tkc, sgq)

                sigs = {}

                def ychain_tanh(i):
                    # ACT leg of the chain, emitted early (inputs 2 old)
                    twc, tkc, sgq = pres.pop(i)
                    sig = wb.tile([P, D], bf16, tag="sig", bufs=3)
                    nc.scalar.activation(sig[:], sgq[:], AF.Tanh, scale=0.5)
                    sigs[i] = (twc, tkc, sig)

                def ychain_dve(i):
                    # y2 = sigmoid(q)*kvc/wc with sigmoid(q) =
                    # 0.5*(tanh(q/2)+1); the 0.5 hides in the w doubling
                    twc, tkc, sig = sigs.pop(i)
                    rec = wb.tile([P, D], bf16, tag="rec", bufs=3)
                    with nc.allow_low_precision(reason="y denominators are bf16 anyway"):
                        nc.vector.reciprocal(rec[:], twc[:])
                    yt = wb.tile([P, D], bf16, tag="yt", bufs=3)
                    nc.vector.tensor_mul(yt[:], tkc[:], rec[:])
                    y2 = wb.tile([P, D], bf16, tag="y2", bufs=3)
                    nc.vector.scalar_tensor_tensor(
                        y2[:], sig[:], 1.0, yt[:],
                        mybir.AluOpType.add, mybir.AluOpType.mult,
                    )
                    y2T = wb.tile([P, 8, P], bf16, tag="y2T", bufs=3)
                    nc.sync.dma_start_transpose(y2T[:], y2[:])
                    mids[i] = y2T

                def ychain_tail(i):
                    y2T = mids.pop(i)
                    if not use_fp8:
                        tails[i] = y2T
                        return
                    y2T8 = wb.tile([P, 8, P], f8, tag="y2T8", bufs=3)
                    nc.scalar.copy(y2T8[:], y2T[:])
                    tails[i] = y2T8

                for j0 in range(min(3, n_tiles)):
                    emit_loads(j0)
                if n_tiles > 0:
                    ychain_pre(0)
                    ychain_tanh(0)
                    ychain_dve(0)
                    ychain_tail(0)
                if n_tiles > 1:
                    ychain_pre(1)
                    ychain_tanh(1)
                    ychain_dve(1)
                hT8s = {}  # ip -> hT8: out mms run two iterations later
                for i in range(n_tiles + 2):
                    if i + 3 < n_tiles:
                        emit_loads(i + 3)
                    if i + 2 < n_tiles:
                        ychain_pre(i + 2)

                    # --- PE: out mms for tile i-2 (inputs two tiles old) ---
                    ip = i - 2
                    if ip in hT8s:
                        hT8p = hT8s.pop(ip)
                        op = [
                            ps_o.tile([P, H], fp32, tag="op", name=f"op{n}")
                            for n in range(2)
                        ]
                        nk = 4 if use_fp8 else 8
                        for m in range(nk):
                            ms = slice(2 * m, 2 * m + 2) if use_fp8 else m
                            for n in range(2):
                                nc.tensor.matmul(
                                    op[n][:], lhsT=hT8p[:, ms, :],
                                    rhs=wout_sb[:, ms, n * H : (n + 1) * H],
                                    start=(m == 0), stop=(m == nk - 1),
                                    perf_mode=DR,
                                )

                    # --- PE: swiglu mms for tile i (inputs one tile old) ---
                    if i < n_tiles:
                        cur = tails.pop(i)
                        pairs = []
                        for j in range(2):
                            pu = ps_uv.tile([P, H], fp32, tag="uv", name=f"uv_u{j}")
                            pg = ps_uv.tile([P, H], fp32, tag="uv", name=f"uv_g{j}")
                            nk = 4 if use_fp8 else 8
                            for m in range(nk):
                                ms = slice(2 * m, 2 * m + 2) if use_fp8 else m
                                nc.tensor.matmul(
                                    pu[:], lhsT=cur[:, ms, :],
                                    rhs=wsw_sb[:, ms, j * H : (j + 1) * H],
                                    start=(m == 0), stop=(m == nk - 1),
                                    perf_mode=DR,
                                )
                                nc.tensor.matmul(
                                    pg[:], lhsT=cur[:, ms, :],
                                    rhs=wsw_sb[:, ms, (2 + j) * H : (3 + j) * H],
                                    start=(m == 0), stop=(m == nk - 1),
                                    perf_mode=DR,
                                )
                            pairs.append((pu, pg))

                    # --- residual + store for i-2 (op frees early) ---
                    if ip >= 0 and ip < n_tiles:
                        xt2 = wb.tile([P, D], fp32, tag="xt2")
                        for n in range(2):
                            ns = slice(n * H, (n + 1) * H)
                            # op is 512x (h 32x, w_out 16x); residual comes
                            # from the bf16 x stash, no re-read
                            nc.vector.scalar_tensor_tensor(
                                xt2[:, ns], op[n][:], 1.0 / 512.0,
                                x_sb[:, ip, ns],
                                mybir.AluOpType.mult, mybir.AluOpType.add,
                            )
                        nc.sync.dma_start(out_t[ip], xt2[:])

                    # --- ACT legs that are ready now, then DVE chain ---
                    if i + 2 < n_tiles:
                        ychain_tanh(i + 2)
                    if i + 1 < n_tiles:
                        ychain_tail(i + 1)
                    if i + 2 < n_tiles:
                        ychain_dve(i + 2)

                    # --- this tile's silu/h (frees the uv psums) ---
                    if i < n_tiles:
                        h = wb.tile([P, D], bf16, tag="h", bufs=2)
                        for j, (pu, pg) in enumerate(pairs):
                            js = slice(j * H, (j + 1) * H)
                            # weights are host-scaled 16x, y2 carries 32x
                            # (16x weights + the 2x w-doubling), so pg/pu
                            # are 512x; h is kept at 32x so its fp8 cast
                            # stays clear of e4m3 subnormals yet under 448
                            sl = wb.tile([P, H], bf16, tag="sl", name=f"sl{j}", bufs=3)
                            nc.scalar.activation(
                                sl[:], pg[:], AF.Silu, scale=1.0 / 512.0
                            )
                            nc.vector.scalar_tensor_tensor(
                                h[:, js], pu[:], 1.0 / 16.0, sl[:],
                                mybir.AluOpType.mult, mybir.AluOpType.mult,
                            )
                        hT = wb.tile([P, 8, P], bf16, tag="hT")
                        nc.sync.dma_start_transpose(hT[:], h[:])
                        if use_fp8:
                            hT8 = wb.tile([P, 8, P], f8, tag="hT8", bufs=4)
                            nc.gpsimd.tensor_copy(hT8[:], hT[:])
                        else:
                            hT8 = hT
                        hT8s[i] = hT8

    nc.compile()
    return nc


def _host_inputs(x, w_qkv, w_swiglu, w_out, use_fp8=True):
    bf = ml_dtypes.bfloat16
    f8 = ml_dtypes.float8_e4m3fn if use_fp8 else bf
    # weights are ~uniform(+-0.054): scale 16x so the fp8 cast stays out of
    # e4m3's subnormal range (min normal 2^-6).  q/k are rms-normalized so
    # the scale cancels; the v->kv->y2 chain carries it (y2 16x helps its
    # own fp8 cast); phase B undoes 256x/2048x in fused scalar ops.
    wqkvT = np.ascontiguousarray(w_qkv.T * 16.0).astype(f8)
    wswT = np.ascontiguousarray(w_swiglu.T * 16.0).astype(f8)
    woutT = np.ascontiguousarray(w_out.T * 16.0).astype(f8)
    tri = np.triu(np.ones((P, P), np.float32)).astype(bf)
    in_maps = []
    for c in range(N_CORES):
        b, h = c // 2, c % 2
        in_maps.append(
            {
                "x": np.ascontiguousarray(
                    x[b, h * CHUNK : (h + 1) * CHUNK, :]
                ).astype(np.float32),
                "wqkvT": wqkvT,
                "wswT": wswT,
                "woutT": woutT,
                "triT": tri,
                "cmask": np.full((2, 1), float(h), np.float32),
            }
        )
    return in_maps


def kernel(x, w_qkv, w_swiglu, w_out, trace=False):
    from concourse.bass_utils import run_bass_kernel_spmd

    x = np.asarray(x, dtype=np.float32)
    w_qkv = np.asarray(w_qkv, dtype=np.float32)
    w_swiglu = np.asarray(w_swiglu, dtype=np.float32)
    w_out = np.asarray(w_out, dtype=np.float32)

    key = "full"
    if key not in _nc_cache:
        _nc_cache[key] = build_nc(NT_FULL, N_CORES, use_collective=True, use_fp8=USE_FP8)
    nc = _nc_cache[key]

    in_maps = _host_inputs(x, w_qkv, w_swiglu, w_out, use_fp8=USE_FP8)
    res = run_bass_kernel_spmd(
        nc, in_maps, core_ids=list(range(N_CORES)), trace=trace
    )
    out = np.empty((B_FULL, T_FULL, D), np.float32)
    for c in range(N_CORES):
        b, h = c // 2, c % 2
        out[b, h * CHUNK : (h + 1) * CHUNK, :] = res.results[c]["out"]
    kernel.last_result = res
    return out

